# revision 4
# baseline (speedup 1.0000x reference)
"""MLA (multi-head latent attention) Bass kernel for Trainium2, 8 NeuronCores.

Strategy: data-parallel over batch (B=8 -> one batch element per core).
All matmuls run as float32r (full-rate fp32, ~1.6e-4 rel err per matmul).
All activations are kept "transposed" (feature dim on partitions) so that no
on-device transposes are needed except the attention-probability transpose,
which runs on the PE via identity matmul.

Host-side prep (free, off-device): transpose inputs/weights, pre-block wq /
wdkv / wkr / wupk for 8KB-contiguous DMA descriptors, build rope cos/sin
tables (transposed, sign-folded).

Per-core pipeline:
  qT[h]   = wqT_blk[h] . xqT          (16 heads x 16 K-chunks)
  ckvT[l] = wdkvT_blk[l] . xkvT       (4 chunks x 16)
  v[t,od] = ckvT . wupvT              (N-layout, tokens on partitions)
  per head-pair: krT = wkrT_blk . xkvT ; knT = wupkT_blk . ckvT
     rope via SBUF->SBUF DMA partition shifts + DVE FMA -> kT[h]
  per head: scores = qT[h]^T . kT[h] -> exp(ACT, accum row sums)
     -> attn = E / rowsum (DVE) -> DMA out; PE-transpose attn -> aT
     -> ctxT[h] = v^T . aT
  out = ctxT^T . woT  (N-layout) -> DMA out
"""

import numpy as np

B = 8
S = 512
D_MODEL = 2048
H = 16
DK = 128
DL = 512
R = 64
ROPE_BASE = 10000.0
SCALE = 1.0 / np.sqrt(np.float32(DK))

_CACHE = {}


def _build_program():
    import concourse.mybir as mybir
    import concourse.tile as tile
    from concourse import bacc

    f32 = mybir.dt.float32
    f32r = mybir.dt.float32r

    nc = bacc.Bacc(None, target_bir_lowering=False)

    # ---- DRAM I/O ----
    xq_d = nc.dram_tensor("xqT", [16, 128, S], f32r, kind="ExternalInput")
    xkv_d = nc.dram_tensor("xkvT", [16, 128, S], f32r, kind="ExternalInput")
    wq_d = nc.dram_tensor("wq_blk", [16, 128, 16, 128], f32r, kind="ExternalInput")
    wdkv_d = nc.dram_tensor("wdkv_blk", [4, 128, 16, 128], f32r, kind="ExternalInput")
    wkr_d = nc.dram_tensor("wkr_blk", [8, 128, 16, 128], f32r, kind="ExternalInput")
    wupk_d = nc.dram_tensor("wupk_blk", [8, 128, 4, 128], f32r, kind="ExternalInput")
    wupv_d = nc.dram_tensor("wupvT", [DL, D_MODEL], f32r, kind="ExternalInput")
    wo_d = nc.dram_tensor("woT", [D_MODEL, D_MODEL], f32r, kind="ExternalInput")
    cos_d = nc.dram_tensor("cosT", [R, S], f32, kind="ExternalInput")
    sin_d = nc.dram_tensor("sinST", [R, S], f32, kind="ExternalInput")
    id_d = nc.dram_tensor("ident", [128, 128], f32, kind="ExternalInput")
    out_d = nc.dram_tensor("out", [S, D_MODEL], f32, kind="ExternalOutput")
    attn_d = nc.dram_tensor("attn", [H, S, S], f32, kind="ExternalOutput")

    Exp = mybir.ActivationFunctionType.Exp

    with tile.TileContext(nc) as tc:
        with (
            tc.tile_pool(name="cn", bufs=1) as cn,
            tc.tile_pool(name="sb", bufs=64) as sb,
            tc.tile_pool(name="wb", bufs=3) as wb,
            tc.tile_pool(name="hf", bufs=3) as hf,
            tc.tile_pool(name="sm", bufs=8) as sm,
            tc.tile_pool(name="ps", bufs=2, space="PSUM") as ps,
        ):
            # ---- constants ----
            cos_t = cn.tile([R, S], f32, tag="cos")
            sin_t = cn.tile([R, S], f32, tag="sin")
            id_t = cn.tile([128, 128], f32, tag="id")
            nc.sync.dma_start(cos_t[:], cos_d[:])
            nc.sync.dma_start(sin_t[:], sin_d[:])
            nc.sync.dma_start(id_t[:], id_d[:])

            # ---- inputs ----
            xq = []
            xkv = []
            for ki in range(16):
                t = sb.tile([128, S], f32r, tag="sb512")
                nc.sync.dma_start(t[:], xq_d[ki])
                xq.append(t)
            for ki in range(16):
                t = sb.tile([128, S], f32r, tag="sb512")
                nc.sync.dma_start(t[:], xkv_d[ki])
                xkv.append(t)

            # ---- q projection: qT[h] [128(dk), 512(tok)] ----
            qT = []
            for h in range(H):
                wt = wb.tile([128, 16, 128], f32r, tag="wblk")
                nc.sync.dma_start(wt[:], wq_d[h])
                p = ps.tile([128, S], f32, tag="prod")
                for ki in range(16):
                    nc.tensor.matmul(p[:], wt[:, ki, :], xq[ki][:],
                                     start=(ki == 0), stop=(ki == 15))
                t = sb.tile([128, S], f32r, tag="sb512")
                nc.vector.tensor_copy(t[:], p[:])
                qT.append(t)

            # ---- c_kv projection: ckvT[l] [128(lat), 512(tok)] ----
            ckvT = []
            for l in range(4):
                wt = wb.tile([128, 16, 128], f32r, tag="wblk")
                nc.sync.dma_start(wt[:], wdkv_d[l])
                p = ps.tile([128, S], f32, tag="prod")
                for ki in range(16):
                    nc.tensor.matmul(p[:], wt[:, ki, :], xkv[ki][:],
                                     start=(ki == 0), stop=(ki == 15))
                t = sb.tile([128, S], f32r, tag="sb512")
                nc.vector.tensor_copy(t[:], p[:])
                ckvT.append(t)

            # ---- k: rope + nope per head pair -> kT[h] [128(dk), 512] ----
            kT = [sb.tile([128, S], f32r, tag="sb512", name=f"kT{i}")
                  for i in range(H)]
            for pr in range(8):
                # rope projection for heads (2pr, 2pr+1): [128, 512] psum
                wt = wb.tile([128, 16, 128], f32r, tag="wblk")
                nc.sync.dma_start(wt[:], wkr_d[pr])
                p = ps.tile([128, S], f32, tag="prod")
                for ki in range(16):
                    nc.tensor.matmul(p[:], wt[:, ki, :], xkv[ki][:],
                                     start=(ki == 0), stop=(ki == 15))
                r2 = sb.tile([128, S], f32, tag="sb512")
                nc.vector.tensor_copy(r2[:], p[:])

                # nope projection (dims 64..128 of each head): [128, 512] psum
                wn = sb.tile([128, 4, 128], f32r, tag="sb512")
                nc.sync.dma_start(wn[:], wupk_d[pr])
                pn = ps.tile([128, S], f32, tag="prod")
                for l in range(4):
                    nc.tensor.matmul(pn[:], wn[:, l, :], ckvT[l][:],
                                     start=(l == 0), stop=(l == 3))
                # odd head nope: psum parts 64..127 -> kT parts 64..127 (aligned)
                nc.vector.tensor_copy(kT[2 * pr + 1][64:128, :], pn[64:128, :])
                # even head nope: stage + DMA partition shift +64
                n2 = sb.tile([128, S], f32r, tag="sb512")
                nc.vector.tensor_copy(n2[0:64, :], pn[0:64, :])
                nc.sync.dma_start(kT[2 * pr][64:128, :], n2[0:64, :])

                for sub in range(2):
                    h = 2 * pr + sub
                    off = 64 * sub
                    if sub == 0:
                        a_t = None  # read rope input directly from r2[0:64]
                    else:
                        a_t = hf.tile([64, S], f32, tag="ha")
                        nc.sync.dma_start(a_t[:], r2[64:128, :])
                    b_t = hf.tile([64, S], f32, tag="hb")
                    nc.sync.dma_start(b_t[0:32, :], r2[off + 32:off + 64, :])
                    nc.sync.dma_start(b_t[32:64, :], r2[off:off + 32, :])
                    rp = hf.tile([64, S], f32, tag="hr")
                    tmb = hf.tile([64, S], f32, tag="ht")
                    src_a = r2[0:64, :] if sub == 0 else a_t[:]
                    nc.vector.tensor_mul(rp[:], src_a, cos_t[:])
                    nc.vector.tensor_mul(tmb[:], b_t[:], sin_t[:])
                    nc.vector.tensor_add(rp[:], rp[:], tmb[:])
                    nc.vector.tensor_copy(kT[h][0:64, :], rp[:])

            # ---- v projection (N-layout): v[t, od] [128(tok), 512(dims)] ----
            vt = {}
            for od in range(4):
                wv = []
                for l in range(4):
                    t = sb.tile([128, S], f32r, tag="sb512")
                    nc.sync.dma_start(
                        t[:], wupv_d[l * 128:(l + 1) * 128,
                                     od * 512:(od + 1) * 512])
                    wv.append(t)
                for tq in range(4):
                    p = ps.tile([128, S], f32, tag="prod")
                    for l in range(4):
                        nc.tensor.matmul(
                            p[:], ckvT[l][:, tq * 128:(tq + 1) * 128],
                            wv[l][:], start=(l == 0), stop=(l == 3))
                    t = sb.tile([128, S], f32r, tag="sb512")
                    nc.vector.tensor_copy(t[:], p[:])
                    vt[(tq, od)] = t

            # ---- attention per head ----
            ctxT = []
            for h in range(H):
                aT = [sb.tile([128, S], f32r, tag="sb512", name=f"aT{h}_{i}")
                      for i in range(4)]
                for qc in range(4):
                    psc = ps.tile([128, S], f32, tag="scores")
                    nc.tensor.matmul(psc[:], qT[h][:, qc * 128:(qc + 1) * 128],
                                     kT[h][:], start=True, stop=True)
                    E = sb.tile([128, S], f32, tag="sb512")
                    ssum = sm.tile([128, 1], f32, tag="ssum")
                    nc.scalar.activation(E[:], psc[:], Exp,
                                         scale=float(SCALE), accum_out=ssum[:])
                    rinv = sm.tile([128, 1], f32, tag="rinv")
                    nc.vector.reciprocal(rinv[:], ssum[:])
                    at_t = sb.tile([128, S], f32, tag="sb512")
                    nc.vector.tensor_scalar_mul(at_t[:], E[:], rinv[:])
                    nc.sync.dma_start(
                        attn_d[h, qc * 128:(qc + 1) * 128, :], at_t[:])
                    for kc in range(4):
                        pt = ps.tile([128, 128], f32, tag="transp")
                        nc.tensor.transpose(
                            pt[:], at_t[:, kc * 128:(kc + 1) * 128], id_t[:])
                        nc.scalar.copy(aT[kc][:, qc * 128:(qc + 1) * 128],
                                       pt[:])
                pc = ps.tile([128, S], f32, tag="ctx")
                for kc in range(4):
                    nc.tensor.matmul(
                        pc[:], vt[(kc, h // 4)][:, (h % 4) * 128:(h % 4 + 1) * 128],
                        aT[kc][:], start=(kc == 0), stop=(kc == 3))
                t = sb.tile([128, S], f32r, tag="sb512")
                nc.vector.tensor_copy(t[:], pc[:])
                ctxT.append(t)

            # ---- output projection (N-layout) ----
            for od in range(4):
                wo = []
                for hh in range(16):
                    t = sb.tile([128, S], f32r, tag="sb512")
                    nc.sync.dma_start(
                        t[:], wo_d[hh * 128:(hh + 1) * 128,
                                   od * 512:(od + 1) * 512])
                    wo.append(t)
                for tq in range(4):
                    p = ps.tile([128, S], f32, tag="prod")
                    for hh in range(16):
                        nc.tensor.matmul(
                            p[:], ctxT[hh][:, tq * 128:(tq + 1) * 128],
                            wo[hh][:], start=(hh == 0), stop=(hh == 15))
                    o_t = sb.tile([128, S], f32, tag="sb512")
                    nc.vector.tensor_copy(o_t[:], p[:])
                    nc.sync.dma_start(
                        out_d[tq * 128:(tq + 1) * 128,
                              od * 512:(od + 1) * 512], o_t[:])

    nc.finalize()
    return nc


def _host_prep(inputs):
    """Shared (weight) arrays + per-core (activation) arrays."""
    wq = np.ascontiguousarray(np.asarray(inputs["wq_w"], np.float32).T)
    wdkv = np.ascontiguousarray(np.asarray(inputs["wdkv_w"], np.float32).T)
    wupk = np.ascontiguousarray(np.asarray(inputs["wupk_w"], np.float32).T)
    wupv = np.ascontiguousarray(np.asarray(inputs["wupv_w"], np.float32).T)
    wkr = np.ascontiguousarray(np.asarray(inputs["wkr_w"], np.float32).T)
    wo = np.ascontiguousarray(np.asarray(inputs["wo_w"], np.float32).T)

    wq_blk = np.ascontiguousarray(
        wq.reshape(16, 128, 16, 128).transpose(2, 1, 0, 3))
    wdkv_blk = np.ascontiguousarray(
        wdkv.reshape(16, 128, 4, 128).transpose(2, 1, 0, 3))
    wkr_blk = np.ascontiguousarray(
        wkr.reshape(16, 128, 8, 128).transpose(2, 1, 0, 3))
    # wupk: keep only nope dims (64..128 of each head) -> [512, 16*64]
    wupk_nope = wupk.reshape(DL, H, DK)[:, :, R:].reshape(DL, H * (DK - R))
    wupk_blk = np.ascontiguousarray(
        wupk_nope.reshape(4, 128, 8, 128).transpose(2, 1, 0, 3))

    # rope tables (transposed, sin sign-folded for the rotate-half shift)
    inv_freq = 1.0 / (ROPE_BASE ** (np.arange(0, R, 2, dtype=np.float32) / R))
    t = np.arange(S, dtype=np.float32)
    freqs = np.einsum("i,j->ij", t, inv_freq)
    emb = np.concatenate([freqs, freqs], axis=-1)  # [S, R]
    cosT = np.ascontiguousarray(np.cos(emb).T.astype(np.float32))  # [R, S]
    sinT = np.sin(emb).T.astype(np.float32)
    sinS = sinT.copy()
    sinS[: R // 2] = -sinS[: R // 2]
    sinS = np.ascontiguousarray(sinS)
    ident = np.eye(128, dtype=np.float32)

    shared = {
        "wq_blk": wq_blk, "wdkv_blk": wdkv_blk, "wkr_blk": wkr_blk,
        "wupk_blk": wupk_blk, "wupvT": wupv, "woT": wo,
        "cosT": cosT, "sinST": sinS, "ident": ident,
    }

    q = np.asarray(inputs["query_input"], np.float32)
    kv = np.asarray(inputs["kv_input"], np.float32)
    in_maps = []
    for b in range(B):
        m = dict(shared)
        m["xqT"] = np.ascontiguousarray(q[b].T).reshape(16, 128, S)
        m["xkvT"] = np.ascontiguousarray(kv[b].T).reshape(16, 128, S)
        in_maps.append(m)
    return in_maps


def kernel(**inputs):
    from concourse.bass_utils import run_bass_kernel_spmd

    if "nc" not in _CACHE:
        _CACHE["nc"] = _build_program()
    nc = _CACHE["nc"]

    in_maps = _host_prep(inputs)
    res = run_bass_kernel_spmd(nc, in_maps, core_ids=list(range(B)))

    out = np.empty((B, S, D_MODEL), np.float32)
    attn = np.empty((B, H, S, S), np.float32)
    for b in range(B):
        out[b] = res.results[b]["out"]
        attn[b] = res.results[b]["attn"]
    return out, attn
